# revision 1
# baseline (speedup 1.0000x reference)
"""Depth-aware forward-warp (bilinear splat) + flow add, on 8 trn2 cores.

Dense shifted-window splat, single fused pass, fp16 datapath.

For each active integer offset pair (dx, dy) the contribution is
    v_c(src) * tri(bxr(src)-dx) * tri(byr(src)-dy)  added at dst = src+(dx,dy)
with tri(u) = relu(1-|u|).

Layout per chunk (one y-fifth of one image):
    partitions p = x % 128, free f = (xblk, s), x = xblk*128 + p
    s = y slot (6 halo + 108 rows + 6 halo), YS=120, XB=8 (cols 960..1023 pad)
x-shifts (partition dim) via DMA-copied shifted planes (per dx); y-shifts are
free-dim AP offsets. All 3 splat channels (wx, wy, den) accumulate in PSUM via
fp16 identity matmuls (1 cyc/row vs 4 for fp32), dest restricted to the 108
output rows so 3 channels fit in 6 banks.

Precision: coords bxr/byr stay f32 (cells with den near eps amplify any
weight-support error by O(|flow|)); weights are computed in f32 on ACT and
stored fp16 scaled by 64 (out of the subnormal range, so flush-to-zero can
only kill weights < 1e-6 = eps; fp16 keeps *relative* precision, which
cancels in the acc/den ratio). The x-weight is negated (min(ux-64,0) on DVE),
the y-weight positive (relu on ACT), so acc/den are negative and the
normalize uses is_lt/min against -4096*eps.
"""
import sys
sys.path.insert(0, '/opt/trn_rl_repo')
import numpy as np
import concourse.bacc as bacc
import concourse.mybir as mybir
from concourse.tile import TileContext
from concourse.bass_utils import run_bass_kernel_spmd

AF = mybir.ActivationFunctionType
ALU = mybir.AluOpType
DT = mybir.dt.float32
DT16 = mybir.dt.float16

B, H, W = 16, 540, 960
NCORES = 8
IPC = B // NCORES          # images per core (2)
TH = 5                     # y-fifths per image
CH = IPC * TH              # chunks per core (10)
TR = H // TH               # rows per fifth (108)
HALO = 6                   # max |dy|,|dx| supported by padding
YS = TR + 2 * HALO         # y slots per xblk (120)
XB = 8                     # x blocks (1024 col slots, 960 real)
PP = 128
F = XB * YS                # free els per input plane (960)
FOUT = XB * TR             # free els per output plane (864)
PADVAL = np.float32(500.0)     # tri() of it is exactly 0; 64x scale stays finite in fp16
WS = 64.0                  # weight pre-scale; den accumulates 4096x
EPS = np.float32(4096e-6)

_CACHE = {}


def _marshal(plane_b, fifth, pad, dtype=np.float16):
    """plane_b: [H, W] -> [128, F] chunk plane for given y-fifth."""
    out = np.full((YS, XB * PP), pad, dtype=dtype)
    r0 = fifth * TR - HALO
    lo, hi = max(0, r0), min(H, r0 + YS)
    out[lo - r0:hi - r0, :W] = plane_b[lo:hi, :].astype(dtype)
    # [s, xblk, p] -> [p, xblk, s]
    return np.ascontiguousarray(out.reshape(YS, XB, PP).transpose(2, 1, 0).reshape(PP, F))


def _marshal_out(plane_b, fifth):
    """plane_b rows of this fifth -> [128, FOUT] f32 (no halo)."""
    out = np.zeros((TR, XB * PP), dtype=np.float32)
    out[:, :W] = plane_b[fifth * TR:(fifth + 1) * TR, :]
    return np.ascontiguousarray(out.reshape(TR, XB, PP).transpose(2, 1, 0).reshape(PP, FOUT))


def _unmarshal(chunk, fifth, img_plane):
    """chunk [128, FOUT] -> write rows of img_plane [H, W]."""
    t = chunk.reshape(PP, XB, TR).transpose(2, 1, 0).reshape(TR, XB * PP)
    img_plane[fifth * TR:(fifth + 1) * TR, :] = t[:, :W]


def _active_sets(bxr, byr):
    """Per chunk-slot active (dx -> sorted dys), unioned across cores.

    bxr/byr: [B, H, W] displacements (already fp16-rounded)."""
    sets = [dict() for _ in range(CH)]
    for b in range(B):
        k_img = b % IPC
        for t in range(TH):
            k = k_img * TH + t
            lo, hi = max(0, t * TR - HALO), min(H, t * TR + TR + HALO)
            fx = np.floor(bxr[b, lo:hi]).astype(np.int64)
            fy = np.floor(byr[b, lo:hi]).astype(np.int64)
            pairs = set()
            for ox in (0, 1):
                for oy in (0, 1):
                    h2 = np.unique((fx + ox) * 1000 + (fy + oy))
                    pairs.update(int(v) for v in h2)
            d = sets[k]
            for v in pairs:
                dx, dy = v // 1000, v % 1000
                if dy > 500:
                    dy -= 1000
                    dx += 1
                if abs(dx) > HALO or abs(dy) > HALO:
                    raise ValueError(f"displacement out of range: dx={dx} dy={dy}")
                d.setdefault(dx, set()).add(dy)
    return tuple(
        tuple(sorted((dx, tuple(sorted(dys))) for dx, dys in s.items()))
        for s in sets
    )


def _register_consts(nc):
    vals = {1.0, 0.0, float(WS)}
    for d in range(-HALO, HALO + 1):
        vals.add(float(-d))
        vals.add(float(-WS * d))
    for v in vals:
        key = (DT, float(v))
        if key in nc.const_aps.aps:
            continue
        t = nc.alloc_sbuf_tensor(f"constf32_{v}", [PP, 1], DT)
        nc.gpsimd.memset(t.ap(), float(v))
        nc.const_aps.aps[key] = t.ap()


def _shift_x(nc, dst, src, dx, zeros):
    """dst[p, xb, s] = src[(p-dx) mod..., xb-carry, s], zero where no source."""
    a = abs(dx)
    if dx > 0:
        nc.sync.dma_start(out=dst[dx:PP, :, :], in_=src[0:PP - dx, :, :])
        nc.sync.dma_start(out=dst[0:dx, 1:XB, :], in_=src[PP - dx:PP, 0:XB - 1, :])
        nc.sync.dma_start(out=dst[0:dx, 0:1, :], in_=zeros[0:dx, :, :])
    else:
        nc.sync.dma_start(out=dst[0:PP - a, :, :], in_=src[a:PP, :, :])
        nc.sync.dma_start(out=dst[PP - a:PP, 0:XB - 1, :], in_=src[0:a, 1:XB, :])
        nc.sync.dma_start(out=dst[PP - a:PP, XB - 1:XB, :], in_=zeros[PP - a:PP, :, :])


def build_program(active, reps=1, n_cores=NCORES):
    nc = bacc.Bacc(trn_type="TRN2", debug=False, num_devices=n_cores)
    _register_consts(nc)

    def param16(name):
        return nc.declare_dram_parameter(name, [CH, PP, F], DT16, isOutput=False)

    bxr_d = nc.declare_dram_parameter("bxr", [CH, PP, F], DT, isOutput=False)
    byr_d = nc.declare_dram_parameter("byr", [CH, PP, F], DT, isOutput=False)
    fbx_d, fby_d, dep_d = param16("fbx"), param16("fby"), param16("dep")
    fax_d = nc.declare_dram_parameter("fax", [CH, PP, FOUT], DT, isOutput=False)
    fay_d = nc.declare_dram_parameter("fay", [CH, PP, FOUT], DT, isOutput=False)
    outx_d = nc.declare_dram_parameter("outx", [CH, PP, FOUT], DT, isOutput=True)
    outy_d = nc.declare_dram_parameter("outy", [CH, PP, FOUT], DT, isOutput=True)
    ident_d = nc.declare_dram_parameter("ident", [PP, PP], DT16, isOutput=False)

    with TileContext(nc) as tc:
        from contextlib import ExitStack
        with ExitStack() as ctx:
            z_pool = ctx.enter_context(tc.tile_pool(name="z", bufs=1))
            zeros = z_pool.tile([PP, 1, YS], DT16, tag="zeros", name="zeros_t")
            nc.vector.memset(zeros[:], 0.0)
            zeros32 = z_pool.tile([PP, 1, YS], DT, tag="zeros32", name="zeros32_t")
            nc.vector.memset(zeros32[:], 0.0)
            ident = z_pool.tile([PP, PP], DT16, tag="ident", name="ident_t")
            nc.sync.dma_start(out=ident[:], in_=ident_d[:])
            psum_pool = ctx.enter_context(tc.tile_pool(name="psum", bufs=1, space="PSUM"))
            io_pool = ctx.enter_context(tc.tile_pool(name="io", bufs=1))
            v_pool = ctx.enter_context(tc.tile_pool(name="v", bufs=1))
            acc_pool = ctx.enter_context(tc.tile_pool(name="acc", bufs=1))
            sh_pool = ctx.enter_context(tc.tile_pool(name="sh", bufs=4))
            w_pool = ctx.enter_context(tc.tile_pool(name="w", bufs=2))
            w2_pool = ctx.enter_context(tc.tile_pool(name="w2", bufs=4))
            tm_pool = ctx.enter_context(tc.tile_pool(name="tm", bufs=4))
            fin_pool = ctx.enter_context(tc.tile_pool(name="fin", bufs=1))

            SH3 = [PP, XB, YS]

            def chunk_body(k):
                bxr = io_pool.tile(SH3, DT, tag="bxr", name="bxr_t")
                byr = io_pool.tile(SH3, DT, tag="byr", name="byr_t")
                fbx = io_pool.tile(SH3, DT16, tag="fbx", name="fbx_t")
                fby = io_pool.tile(SH3, DT16, tag="fby", name="fby_t")
                dep = io_pool.tile(SH3, DT16, tag="dep", name="dep_t")
                for t, d in ((bxr, bxr_d), (byr, byr_d), (fbx, fbx_d),
                             (fby, fby_d), (dep, dep_d)):
                    nc.sync.dma_start(out=t[:], in_=d[k].rearrange("p (xb s) -> p xb s", xb=XB))

                # v2 = exp(-dep); v0 = fbx*v2; v1 = fby*v2
                v0 = v_pool.tile(SH3, DT16, tag="v0", name="v0_t")
                v1 = v_pool.tile(SH3, DT16, tag="v1", name="v1_t")
                v2 = v_pool.tile(SH3, DT16, tag="v2", name="v2_t")
                nc.scalar.activation(v2[:], dep[:], AF.Exp, bias=0.0, scale=-1.0)
                nc.vector.tensor_mul(v0[:], fbx[:], v2[:])
                nc.vector.tensor_mul(v1[:], fby[:], v2[:])
                vs = (v0, v1, v2)

                # 3 channels x [128, 2 bank-groups, 4 xblks, 128 slots] PSUM
                psums = [psum_pool.tile([PP, 2, 4, 128], DT, tag=f"psum{c}",
                                        name=f"psum{c}_t") for c in range(3)]
                ncomb = sum(len(dys) for _, dys in active[k])
                ci = 0
                for dx, dys in active[k]:
                    ux = w_pool.tile(SH3, DT, tag="ux", name="ux_t")
                    tx = w_pool.tile(SH3, DT16, tag="tx", name="tx_t")
                    # ux = 64|bxr - dx| in f32 (exact support); tx = min(ux-64, 0)
                    # = -64*relu(1-|bxr-dx|), fp16 (scaled out of subnormals).
                    # x and y weights are both negated+scaled; product is +4096w.
                    nc.scalar.activation(ux[:], bxr[:], AF.Abs, bias=float(-WS * dx), scale=WS)
                    nc.vector.tensor_scalar(tx[:], ux[:], WS, 0.0,
                                            ALU.subtract, ALU.min)
                    ps = [tm_pool.tile(SH3, DT16, tag=f"p{c}", name=f"p{c}_t")
                          for c in range(3)]
                    nc.vector.tensor_mul(ps[0][:], v0[:], tx[:])
                    nc.vector.tensor_mul(ps[1][:], v1[:], tx[:])
                    nc.vector.tensor_mul(ps[2][:], v2[:], tx[:])
                    if dx == 0:
                        pss, byrs = ps, byr
                    else:
                        pss = [sh_pool.tile(SH3, DT16, tag=f"ps{c}", name=f"ps{c}_t")
                               for c in range(3)]
                        byrs = sh_pool.tile(SH3, DT, tag="byrs", name="byrs_t")
                        for c in range(3):
                            _shift_x(nc, pss[c], ps[c], dx, zeros)
                        _shift_x(nc, byrs, byr, dx, zeros32)
                    for dy in dys:
                        uy = w2_pool.tile(SH3, DT, tag="uy", name="uy_t")
                        ty = w2_pool.tile(SH3, DT16, tag="ty", name="ty_t")
                        s0 = HALO - dy  # source slot of first output row
                        sl = (slice(None), slice(None), slice(s0, s0 + TR))
                        # uy = 64|byr-dy| (f32); ty = relu(64-uy) = +64*tri (fp16).
                        # tx is negated so acc/den are both negative; ratio is
                        # unchanged and the mask/max ops below are sign-flipped.
                        # Only the TR source slots the matmul reads are computed.
                        nc.scalar.activation(uy[sl], byrs[sl], AF.Abs, bias=float(-WS * dy), scale=WS)
                        nc.scalar.activation(ty[sl], uy[sl], AF.Relu, bias=WS, scale=-1.0)
                        tms = [tm_pool.tile(SH3, DT16, tag=f"tm{c}", name=f"tm{c}_t")
                               for c in range(3)]
                        nc.vector.tensor_mul(tms[0][sl], pss[0][sl], ty[sl])
                        nc.vector.tensor_mul(tms[1][sl], pss[1][sl], ty[sl])
                        nc.vector.tensor_mul(tms[2][sl], pss[2][sl], ty[sl])
                        for c in range(3):
                            for g in range(2):
                                nc.tensor.matmul(
                                    psums[c][:, g:g + 1, :, 0:TR],
                                    ident[:],
                                    tms[c][:, 4 * g:4 * g + 4, s0:s0 + TR],
                                    start=(ci == 0),
                                    stop=(ci == ncomb - 1),
                                )
                        ci += 1

                # drain + normalize + add flowAB
                accs = [acc_pool.tile([PP, 2, 4, TR], DT, tag=f"acc{c}",
                                      name=f"acc{c}_t") for c in range(3)]
                for c in range(3):
                    nc.scalar.copy(accs[c][:], psums[c][:, :, :, 0:TR])
                SHO = [PP, 2, 4, TR]
                fax = io_pool.tile(SHO, DT, tag="fax", name="fax_t")
                fay = io_pool.tile(SHO, DT, tag="fay", name="fay_t")
                nc.sync.dma_start(out=fax[:], in_=fax_d[k].rearrange("p (a b s) -> p a b s", a=2, b=4))
                nc.sync.dma_start(out=fay[:], in_=fay_d[k].rearrange("p (a b s) -> p a b s", a=2, b=4))
                mask = fin_pool.tile(SHO, DT, tag="mask", name="mask_t")
                mx = fin_pool.tile(SHO, DT, tag="mx", name="mx_t")
                rec = fin_pool.tile(SHO, DT, tag="rec", name="rec_t")
                nc.vector.tensor_scalar(mask[:], accs[2][:], float(-EPS), None, ALU.is_lt)
                nc.vector.tensor_scalar(mx[:], accs[2][:], float(-EPS), None, ALU.min)
                nc.vector.reciprocal(rec[:], mx[:])
                for c, (fa, od) in enumerate(((fax, outx_d), (fay, outy_d))):
                    w1 = fin_pool.tile(SHO, DT, tag=f"w1_{c}", name=f"w1_{c}_t")
                    w2 = fin_pool.tile(SHO, DT, tag=f"w2_{c}", name=f"w2_{c}_t")
                    w3 = fin_pool.tile(SHO, DT, tag=f"w3_{c}", name=f"w3_{c}_t")
                    nc.vector.tensor_mul(w1[:], accs[c][:], rec[:])
                    nc.vector.tensor_mul(w2[:], w1[:], mask[:])
                    nc.vector.tensor_add(w3[:], w2[:], fa[:])
                    nc.sync.dma_start(out=od[k].rearrange("p (a b s) -> p a b s", a=2, b=4), in_=w3[:])

            if reps == 1:
                for k in range(CH):
                    chunk_body(k)
            else:
                with tc.For_i(0, reps, 1):
                    for k in range(CH):
                        chunk_body(k)
    nc.finalize()
    return nc


def _prepare(flowAB, back_flowAB, flowBC, imgB_depth):
    """Host marshaling. Returns (active, in_maps)."""
    flowAB = np.asarray(flowAB, dtype=np.float32)
    back = np.asarray(back_flowAB, dtype=np.float32)
    fbc = np.asarray(flowBC, dtype=np.float32)
    dep = np.asarray(imgB_depth, dtype=np.float32)

    xx = np.arange(W, dtype=np.float32)[None, :]
    yy = np.arange(H, dtype=np.float32)[:, None]
    # reproduce reference's fl(x+bx)-x rounding exactly
    bxr = (xx + back[:, 0]) - xx
    byr = (yy + back[:, 1]) - yy
    active = _active_sets(bxr, byr)

    in_maps = []
    for core in range(NCORES):
        m = {n: np.empty((CH, PP, F), np.float16) for n in
             ("fbx", "fby", "dep")}
        m["bxr"] = np.empty((CH, PP, F), np.float32)
        m["byr"] = np.empty((CH, PP, F), np.float32)
        m["fax"] = np.empty((CH, PP, FOUT), np.float32)
        m["fay"] = np.empty((CH, PP, FOUT), np.float32)
        m["ident"] = np.eye(PP, dtype=np.float16)
        for ki in range(IPC):
            b = core * IPC + ki
            for t in range(TH):
                k = ki * TH + t
                m["bxr"][k] = _marshal(bxr[b], t, PADVAL, np.float32)
                m["byr"][k] = _marshal(byr[b], t, PADVAL, np.float32)
                m["fbx"][k] = _marshal(fbc[b, 0], t, 0.0)
                m["fby"][k] = _marshal(fbc[b, 1], t, 0.0)
                m["dep"][k] = _marshal(dep[b, 0], t, 0.0)
                m["fax"][k] = _marshal_out(flowAB[b, 0], t)
                m["fay"][k] = _marshal_out(flowAB[b, 1], t)
        in_maps.append(m)
    return active, in_maps


def kernel(flowAB, back_flowAB, flowBC, imgB_depth):
    active, in_maps = _prepare(flowAB, back_flowAB, flowBC, imgB_depth)
    if active not in _CACHE:
        _CACHE[active] = build_program(active)
    nc = _CACHE[active]
    res = run_bass_kernel_spmd(nc, in_maps, core_ids=list(range(NCORES)))
    out = np.empty((B, 2, H, W), np.float32)
    for core in range(NCORES):
        r = res.results[core]
        for ki in range(IPC):
            b = core * IPC + ki
            for t in range(TH):
                k = ki * TH + t
                _unmarshal(r["outx"][k], t, out[b, 0])
                _unmarshal(r["outy"][k], t, out[b, 1])
    return out


if __name__ == "__main__":
    sys.path.insert(0, '/root/problem')
    import importlib.util
    spec = importlib.util.spec_from_file_location("reference", "/root/problem/reference.py")
    ref = importlib.util.module_from_spec(spec)
    spec.loader.exec_module(ref)
    inputs = {k: np.asarray(v) for k, v in ref.setup_inputs().items()}
    expected = np.asarray(ref.reference(**inputs))
    got = kernel(**inputs)
    err = np.abs(got - expected)
    rel = err.max() / (np.abs(expected).max() + 1e-30)
    print(f"abs max err: {err.max():.3e}  rel: {rel:.3e}")

